# revision 9
# baseline (speedup 1.0000x reference)
"""Bahdanau-attention kernel for TRN2, 8 NeuronCores, batch-parallel.

Per core (batch shard b=32):
  x = img_features [6272, 2048] f32 (bl-flattened)
  K_s^T[h, bl]  = sum_e WkT[e,h] * xT[e,bl]          (PE, bf16, xT via PE transpose)
  += (Qh + bq + bk) via indicator matmul              (PE)
  attT = tanh(psum)                                   (ACT, drains psum -> sbuf bf16)
  e[bl] = Wv . attT                                   (PE, M=1)
  w = exp(e)                                          (ACT; softmax shift-invariance
                                                       makes max-subtraction and bv
                                                       unnecessary: |e| <= ~11)
  ctx = sum_bl w[bl] * x[bl, :] (per batch, via indicator-masked lhsT), then / sum(w)
  alpha = w / sum(w)
"""

import sys

for _p in ("/opt/trn_rl_repo",):
    if _p not in sys.path:
        sys.path.insert(0, _p)

import numpy as np
import ml_dtypes

import concourse.bass as bass
import concourse.mybir as mybir
from concourse import bacc
from concourse.tile import TileContext
from concourse.bass_utils import run_bass_kernel_spmd

BF = ml_dtypes.bfloat16
NCORES = 8
BFULL = 256
B, L, E, H = 32, 196, 2048, 512  # per-core batch shard
BL = B * L                        # 6272
NT = BL // 128                    # 49 x row-tiles
CH = 512                          # bl chunk (free dim of K_s matmuls)
NCH = (BL + CH - 1) // CH         # 13 (12 full + 1 tail of 128)
ET = E // 128                     # 16
HT = H // 128                     # 4
EC = E // 512                     # 4 (ctx free chunks)

f32 = mybir.dt.float32
bf16 = mybir.dt.bfloat16
Act = mybir.ActivationFunctionType
AX = mybir.AxisListType

LAST_RESULT = None
_NC = None


def _build():
    nc = bacc.Bacc()
    x = nc.declare_dram_parameter("x", [BL, E], f32, isOutput=False)
    hT = nc.declare_dram_parameter("hiddenT", [H, B], bf16, isOutput=False)
    wqT = nc.declare_dram_parameter("wqT", [H, H], bf16, isOutput=False)
    wkT = nc.declare_dram_parameter("wkT", [E, H], bf16, isOutput=False)
    bqk = nc.declare_dram_parameter("bqk", [1, H], bf16, isOutput=False)
    wv = nc.declare_dram_parameter("wv", [128, HT], bf16, isOutput=False)
    ind = nc.declare_dram_parameter("ind", [BL, B], bf16, isOutput=False)
    inda = nc.declare_dram_parameter("ind_aug", [B + 1, BL], bf16, isOutput=False)
    ident = nc.declare_dram_parameter("identity", [128, 128], bf16, isOutput=False)
    ctx_o = nc.declare_dram_parameter("ctx", [B, E], f32, isOutput=True)
    alpha_o = nc.declare_dram_parameter("alpha", [B, L], f32, isOutput=True)

    with TileContext(nc) as tc:
        with (
            tc.tile_pool(name="singles", bufs=1) as singles,
            tc.tile_pool(name="xb", bufs=18) as xbp,
            tc.tile_pool(name="xT", bufs=2) as xtp,
            tc.tile_pool(name="attT", bufs=2) as atp,
            tc.tile_pool(name="wt", bufs=8) as wtp,
            tc.tile_pool(name="wsl", bufs=2) as wslp,
            tc.tile_pool(name="wcol", bufs=2) as wcolp,
            tc.tile_pool(name="tail", bufs=1) as tailp,
            tc.tile_pool(name="ppt", bufs=3, space="PSUM") as ppt,
            tc.tile_pool(name="ppk", bufs=2, space="PSUM") as ppk,
            tc.tile_pool(name="ppe", bufs=1, space="PSUM") as ppe,
            tc.tile_pool(name="ppc", bufs=2, space="PSUM") as ppc,
            tc.tile_pool(name="dram", bufs=1, space="DRAM") as dramp,
        ):
            # ---------- constants (small / critical-path first: the sync queue
            # drains slowly against the 1MB x-load packets, so queue order
            # decides when PE can start) ----------
            id_sb = singles.tile([128, 128], bf16)
            nc.sync.dma_start(out=id_sb, in_=ident[:])
            wv_sb = singles.tile([128, HT], bf16)
            nc.sync.dma_start(out=wv_sb, in_=wv[:])
            hT_sb = singles.tile([128, HT, B], bf16)
            nc.sync.dma_start(out=hT_sb, in_=hT[:].rearrange("(t p) b -> p t b", p=128))
            wqT_sb = singles.tile([128, HT, H], bf16)
            nc.sync.dma_start(out=wqT_sb, in_=wqT[:].rearrange("(t p) h -> p t h", p=128))
            ind_sb = singles.tile([128, NT, B], bf16)
            nc.sync.dma_start(out=ind_sb, in_=ind[:].rearrange("(k p) b -> p k b", p=128))
            inda_sb = singles.tile([B + 1, BL], bf16)
            nc.sync.dma_start(out=inda_sb, in_=inda[:])
            wkT_sb = singles.tile([128, ET, H], bf16)
            nc.sync.dma_start(out=wkT_sb, in_=wkT[:].rearrange("(k p) h -> p k h", p=128))

            ctx_sb = singles.tile([B, E], f32)
            nc.vector.memset(ctx_sb, 0.0)
            e_dram = dramp.tile([BL], f32)

            # ---------- Q_h = hidden @ Wq.T (+ bq + bk), layout [b, h] ----------
            psq = ppc.tile([B, 512], f32, tag="pc")
            for t in range(HT):
                nc.tensor.matmul(
                    psq,
                    lhsT=hT_sb[:, t, :],
                    rhs=wqT_sb[:, t, :],
                    start=(t == 0),
                    stop=(t == HT - 1),
                )
            qb_sb = singles.tile([B + 1, H], bf16)
            nc.scalar.copy(out=qb_sb[0:B, :], in_=psq[:, :])
            nc.sync.dma_start(out=qb_sb[B : B + 1, :], in_=bqk[:])

            # ---------- main streaming loop over bl chunks ----------
            for c in range(NCH):
                n0 = c * CH
                n1 = min(BL, n0 + CH)
                n = n1 - n0
                kt = n // 128  # x-tiles in this chunk (4, tail: 1)

                xb = []
                for i in range(kt):
                    t_ = xbp.tile([128, E], bf16, tag="xb")
                    nc.gpsimd.dma_start(
                        out=t_, in_=x[n0 + 128 * i : n0 + 128 * (i + 1), :]
                    )  # f32 -> bf16 cast in SWDGE
                    xb.append(t_)

                # transpose x chunk: xT[e_part, k, bl]. Regular matmul
                # (out = x_tile.T @ I) instead of transpose-mode: pipelines
                # with LDWEIGHTS and keeps the PE clock warm; products are
                # exact (x1.0, f32 psum).
                xT_c = xtp.tile([128, ET, CH], bf16, tag="xT")
                for k in range(ET):
                    pt = ppt.tile([128, CH], f32, tag="pt")
                    for i in range(kt):
                        nc.tensor.matmul(
                            pt[:, 128 * i : 128 * (i + 1)],
                            lhsT=xb[i][:, 128 * k : 128 * (k + 1)],
                            rhs=id_sb,
                        )
                    # drain psum -> sbuf; split between DVE and ACT
                    if k % 3 == 0:
                        nc.scalar.copy(out=xT_c[:, k, :n], in_=pt[:, :n])
                    else:
                        nc.vector.tensor_copy(out=xT_c[:, k, :n], in_=pt[:, :n])

                # K_s^T chunk + QB + tanh
                attT_c = atp.tile([128, HT, CH], bf16, tag="attT")
                for j in range(HT):
                    pk = ppk.tile([128, CH], f32, tag="pk")
                    for k in range(ET):
                        nc.tensor.matmul(
                            pk[:, :n],
                            lhsT=wkT_sb[:, k, 128 * j : 128 * (j + 1)],
                            rhs=xT_c[:, k, :n],
                            start=(k == 0),
                            stop=False,
                        )
                    nc.tensor.matmul(
                        pk[:, :n],
                        lhsT=qb_sb[:, 128 * j : 128 * (j + 1)],
                        rhs=inda_sb[:, n0:n1],
                        start=False,
                        stop=True,
                    )
                    nc.scalar.activation(
                        out=attT_c[:, j, :n], in_=pk[:, :n], func=Act.Tanh
                    )

                # e = Wv . attT ; w = exp(e)
                pe_ = ppe.tile([1, CH], f32, tag="pe")
                for j in range(HT):
                    nc.tensor.matmul(
                        pe_[:, :n],
                        lhsT=wv_sb[:, j : j + 1],
                        rhs=attT_c[:, j, :n],
                        start=(j == 0),
                        stop=(j == HT - 1),
                    )
                wsl = wslp.tile([1, CH], f32, tag="wsl")
                nc.scalar.activation(out=wsl[:, :n], in_=pe_[:, :n], func=Act.Exp)
                nc.sync.dma_start(out=e_dram[n0:n1], in_=wsl[:, :n])

                # w back as per-tile partition columns
                wcol_c = wcolp.tile([128, 4], f32, tag="wcol")
                nc.sync.dma_start(
                    out=wcol_c[:, :kt],
                    in_=e_dram[n0:n1].rearrange("(k p) -> p k", p=128),
                )

                # ctx += sum_bl w * x   (indicator-masked lhsT, M=32)
                wts = []
                for i in range(kt):
                    kg = n0 // 128 + i
                    wt_i = wtp.tile([128, B], bf16, tag="wt")
                    nc.vector.tensor_scalar_mul(
                        wt_i, ind_sb[:, kg, :], wcol_c[:, i : i + 1]
                    )
                    wts.append(wt_i)
                for ec in range(EC):
                    pc = ppc.tile([B, 512], f32, tag="pc")
                    for i in range(kt):
                        nc.tensor.matmul(
                            pc,
                            lhsT=wts[i],
                            rhs=xb[i][:, 512 * ec : 512 * (ec + 1)],
                            start=(i == 0),
                            stop=(i == kt - 1),
                        )
                    nc.vector.tensor_add(
                        ctx_sb[:, 512 * ec : 512 * (ec + 1)],
                        ctx_sb[:, 512 * ec : 512 * (ec + 1)],
                        pc,
                    )

            # ---------- tail: normalize ----------
            w32 = tailp.tile([B, L], f32)
            nc.sync.dma_start(out=w32, in_=e_dram[:].rearrange("(b l) -> b l", b=B))
            s32 = tailp.tile([B, 1], f32)
            nc.vector.reduce_sum(out=s32, in_=w32, axis=AX.X)
            rs = tailp.tile([B, 1], f32)
            nc.vector.reciprocal(out=rs, in_=s32)
            al = tailp.tile([B, L], f32)
            nc.vector.tensor_scalar_mul(al, w32, rs)
            nc.sync.dma_start(out=alpha_o[:], in_=al)
            nc.vector.tensor_scalar_mul(ctx_sb, ctx_sb, rs)
            nc.sync.dma_start(out=ctx_o[:], in_=ctx_sb)

    nc.finalize()
    return nc


def _get_nc():
    global _NC
    if _NC is None:
        _NC = _build()
    return _NC


def _host_prep(inputs):
    img = np.asarray(inputs["img_features"], dtype=np.float32)
    hid = np.asarray(inputs["hidden_state"], dtype=np.float32)
    Wq = np.asarray(inputs["Wq"], dtype=np.float32)
    bq = np.asarray(inputs["bq"], dtype=np.float32)
    Wk = np.asarray(inputs["Wk"], dtype=np.float32)
    bk = np.asarray(inputs["bk"], dtype=np.float32)
    Wv = np.asarray(inputs["Wv"], dtype=np.float32)

    wqT = np.ascontiguousarray(Wq.T).astype(BF)          # [H, H]
    wkT = np.ascontiguousarray(Wk.T).astype(BF)          # [E, H]
    bqk = (bq + bk).astype(BF)[None, :]                  # [1, H]
    wv_h = np.ascontiguousarray(Wv[0].reshape(HT, 128).T).astype(BF)  # [128, HT]
    ident = np.eye(128, dtype=np.float32).astype(BF)
    rows = np.arange(BL) // L
    ind = (rows[:, None] == np.arange(B)[None, :]).astype(np.float32)
    ind_bf = ind.astype(BF)                              # [BL, B]
    inda = np.concatenate([ind.T, np.ones((1, BL), np.float32)], axis=0).astype(BF)

    in_maps = []
    for i in range(NCORES):
        sl = slice(i * B, (i + 1) * B)
        in_maps.append(
            {
                "x": np.ascontiguousarray(img[sl].reshape(BL, E)),
                "hiddenT": np.ascontiguousarray(hid[sl].T).astype(BF),
                "wqT": wqT,
                "wkT": wkT,
                "bqk": bqk,
                "wv": wv_h,
                "ind": ind_bf,
                "ind_aug": inda,
                "identity": ident,
            }
        )
    return in_maps


def kernel(**inputs):
    global LAST_RESULT
    nc = _get_nc()
    in_maps = _host_prep(inputs)
    res = run_bass_kernel_spmd(nc, in_maps, core_ids=list(range(NCORES)))
    LAST_RESULT = res
    ctx = np.concatenate(
        [np.asarray(res.results[i]["ctx"]) for i in range(NCORES)], axis=0
    ).astype(np.float32)
    alpha = np.concatenate(
        [np.asarray(res.results[i]["alpha"]) for i in range(NCORES)], axis=0
    ).astype(np.float32)
    return ctx, alpha


# revision 10
# speedup vs baseline: 1.1149x; 1.1149x over previous
"""Bahdanau-attention kernel for TRN2, 8 NeuronCores, batch-parallel.

Per core (batch shard b=32):
  x = img_features [6272, 2048] f32 (bl-flattened)
  K_s^T[h, bl]  = sum_e WkT[e,h] * xT[e,bl]          (PE, bf16, xT via PE transpose)
  += (Qh + bq + bk) via indicator matmul              (PE)
  attT = tanh(psum)                                   (ACT, drains psum -> sbuf bf16)
  e[bl] = Wv . attT                                   (PE, M=1)
  w = exp(e)                                          (ACT; softmax shift-invariance
                                                       makes max-subtraction and bv
                                                       unnecessary: |e| <= ~11)
  ctx = sum_bl w[bl] * x[bl, :] (per batch, via indicator-masked lhsT), then / sum(w)
  alpha = w / sum(w)
"""

import sys

for _p in ("/opt/trn_rl_repo",):
    if _p not in sys.path:
        sys.path.insert(0, _p)

import numpy as np
import ml_dtypes

import concourse.bass as bass
import concourse.mybir as mybir
from concourse import bacc
from concourse.tile import TileContext
from concourse.bass_utils import run_bass_kernel_spmd

BF = ml_dtypes.bfloat16
NCORES = 8
BFULL = 256
B, L, E, H = 32, 196, 2048, 512  # per-core batch shard
BL = B * L                        # 6272
NT = BL // 128                    # 49 x row-tiles
CH = 512                          # bl chunk (free dim of K_s matmuls)
NCH = (BL + CH - 1) // CH         # 13 (12 full + 1 tail of 128)
ET = E // 128                     # 16
HT = H // 128                     # 4
EC = E // 512                     # 4 (ctx free chunks)

f32 = mybir.dt.float32
bf16 = mybir.dt.bfloat16
Act = mybir.ActivationFunctionType
AX = mybir.AxisListType

LAST_RESULT = None
_NC = None


def _build():
    nc = bacc.Bacc()
    x = nc.declare_dram_parameter("x", [BL, E], f32, isOutput=False)
    hT = nc.declare_dram_parameter("hiddenT", [H, B], bf16, isOutput=False)
    wqT = nc.declare_dram_parameter("wqT", [H, H], bf16, isOutput=False)
    wkT = nc.declare_dram_parameter("wkT", [E, H], bf16, isOutput=False)
    bqk = nc.declare_dram_parameter("bqk", [1, H], bf16, isOutput=False)
    wv = nc.declare_dram_parameter("wv", [128, HT], bf16, isOutput=False)
    ind = nc.declare_dram_parameter("ind", [BL, B], bf16, isOutput=False)
    inda = nc.declare_dram_parameter("ind_aug", [B + 1, BL], bf16, isOutput=False)
    ident = nc.declare_dram_parameter("identity", [128, 128], bf16, isOutput=False)
    ctx_o = nc.declare_dram_parameter("ctx", [B, E], f32, isOutput=True)
    alpha_o = nc.declare_dram_parameter("alpha", [B, L], f32, isOutput=True)

    with TileContext(nc) as tc:
        with (
            tc.tile_pool(name="singles", bufs=1) as singles,
            tc.tile_pool(name="xb", bufs=18) as xbp,
            tc.tile_pool(name="xT", bufs=2) as xtp,
            tc.tile_pool(name="attT", bufs=2) as atp,
            tc.tile_pool(name="wt", bufs=8) as wtp,
            tc.tile_pool(name="wsl", bufs=2) as wslp,
            tc.tile_pool(name="wcol", bufs=2) as wcolp,
            tc.tile_pool(name="tail", bufs=1) as tailp,
            tc.tile_pool(name="ppt", bufs=2, space="PSUM") as ppt,
            tc.tile_pool(name="ppk", bufs=2, space="PSUM") as ppk,
            tc.tile_pool(name="ppe", bufs=2, space="PSUM") as ppe,
            tc.tile_pool(name="ppc", bufs=2, space="PSUM") as ppc,
            tc.tile_pool(name="dram", bufs=1, space="DRAM") as dramp,
        ):
            # ---------- constants (small / critical-path first: the sync queue
            # drains slowly against the 1MB x-load packets, so queue order
            # decides when PE can start) ----------
            id_sb = singles.tile([128, 128], bf16)
            nc.sync.dma_start(out=id_sb, in_=ident[:])
            wv_sb = singles.tile([128, HT], bf16)
            nc.sync.dma_start(out=wv_sb, in_=wv[:])
            hT_sb = singles.tile([128, HT, B], bf16)
            nc.sync.dma_start(out=hT_sb, in_=hT[:].rearrange("(t p) b -> p t b", p=128))
            wqT_sb = singles.tile([128, HT, H], bf16)
            nc.sync.dma_start(out=wqT_sb, in_=wqT[:].rearrange("(t p) h -> p t h", p=128))
            ind_sb = singles.tile([128, NT, B], bf16)
            nc.sync.dma_start(out=ind_sb, in_=ind[:].rearrange("(k p) b -> p k b", p=128))
            inda_sb = singles.tile([B + 1, BL], bf16)
            nc.sync.dma_start(out=inda_sb, in_=inda[:])
            wkT_sb = singles.tile([128, ET, H], bf16)
            nc.sync.dma_start(out=wkT_sb, in_=wkT[:].rearrange("(k p) h -> p k h", p=128))

            ctx_sb = singles.tile([B, E], f32)
            nc.vector.memset(ctx_sb, 0.0)
            e_dram = dramp.tile([BL], f32)

            # ---------- Q_h = hidden @ Wq.T (+ bq + bk), layout [b, h] ----------
            psq = ppc.tile([B, 512], f32, tag="pc")
            for t in range(HT):
                nc.tensor.matmul(
                    psq,
                    lhsT=hT_sb[:, t, :],
                    rhs=wqT_sb[:, t, :],
                    start=(t == 0),
                    stop=(t == HT - 1),
                )
            qb_sb = singles.tile([B + 1, H], bf16)
            nc.scalar.copy(out=qb_sb[0:B, :], in_=psq[:, :])
            nc.sync.dma_start(out=qb_sb[B : B + 1, :], in_=bqk[:])

            # ---------- main streaming loop over bl chunks ----------
            for c in range(NCH):
                n0 = c * CH
                n1 = min(BL, n0 + CH)
                n = n1 - n0
                kt = n // 128  # x-tiles in this chunk (4, tail: 1)

                xb = []
                for i in range(kt):
                    t_ = xbp.tile([128, E], bf16, tag="xb")
                    nc.gpsimd.dma_start(
                        out=t_, in_=x[n0 + 128 * i : n0 + 128 * (i + 1), :]
                    )  # f32 -> bf16 cast in SWDGE
                    xb.append(t_)

                # transpose x chunk: xT[e_part, k, bl]. Regular matmul
                # (out = x_tile.T @ I) instead of transpose-mode: pipelines
                # with LDWEIGHTS and keeps the PE clock warm; products are
                # exact (x1.0, f32 psum).
                xT_c = xtp.tile([128, ET, CH], bf16, tag="xT")
                for k in range(ET):
                    pt = ppt.tile([128, CH], f32, tag="pt")
                    for i in range(kt):
                        nc.tensor.matmul(
                            pt[:, 128 * i : 128 * (i + 1)],
                            lhsT=xb[i][:, 128 * k : 128 * (k + 1)],
                            rhs=id_sb,
                        )
                    # drain psum -> sbuf; split between DVE and ACT
                    if k % 3 == 0:
                        nc.scalar.copy(out=xT_c[:, k, :n], in_=pt[:, :n])
                    else:
                        nc.vector.tensor_copy(out=xT_c[:, k, :n], in_=pt[:, :n])

                # K_s^T chunk + QB + tanh
                attT_c = atp.tile([128, HT, CH], bf16, tag="attT")
                for j in range(HT):
                    pk = ppk.tile([128, CH], f32, tag="pk")
                    for k in range(ET):
                        nc.tensor.matmul(
                            pk[:, :n],
                            lhsT=wkT_sb[:, k, 128 * j : 128 * (j + 1)],
                            rhs=xT_c[:, k, :n],
                            start=(k == 0),
                            stop=False,
                        )
                    nc.tensor.matmul(
                        pk[:, :n],
                        lhsT=qb_sb[:, 128 * j : 128 * (j + 1)],
                        rhs=inda_sb[:, n0:n1],
                        start=False,
                        stop=True,
                    )
                    nc.scalar.activation(
                        out=attT_c[:, j, :n], in_=pk[:, :n], func=Act.Tanh
                    )

                # e = Wv . attT ; w = exp(e)
                pe_ = ppe.tile([1, CH], f32, tag="pe")
                for j in range(HT):
                    nc.tensor.matmul(
                        pe_[:, :n],
                        lhsT=wv_sb[:, j : j + 1],
                        rhs=attT_c[:, j, :n],
                        start=(j == 0),
                        stop=(j == HT - 1),
                    )
                wsl = wslp.tile([1, CH], f32, tag="wsl")
                nc.scalar.activation(out=wsl[:, :n], in_=pe_[:, :n], func=Act.Exp)
                nc.sync.dma_start(out=e_dram[n0:n1], in_=wsl[:, :n])

                # w back as per-tile partition columns
                wcol_c = wcolp.tile([128, 4], f32, tag="wcol")
                nc.sync.dma_start(
                    out=wcol_c[:, :kt],
                    in_=e_dram[n0:n1].rearrange("(k p) -> p k", p=128),
                )

                # ctx += sum_bl w * x   (indicator-masked lhsT, M=32)
                wts = []
                for i in range(kt):
                    kg = n0 // 128 + i
                    wt_i = wtp.tile([128, B], bf16, tag="wt")
                    nc.vector.tensor_scalar_mul(
                        wt_i, ind_sb[:, kg, :], wcol_c[:, i : i + 1]
                    )
                    wts.append(wt_i)
                for ec in range(EC):
                    pc = ppc.tile([B, 512], f32, tag="pc")
                    for i in range(kt):
                        nc.tensor.matmul(
                            pc,
                            lhsT=wts[i],
                            rhs=xb[i][:, 512 * ec : 512 * (ec + 1)],
                            start=(i == 0),
                            stop=(i == kt - 1),
                        )
                    nc.vector.tensor_add(
                        ctx_sb[:, 512 * ec : 512 * (ec + 1)],
                        ctx_sb[:, 512 * ec : 512 * (ec + 1)],
                        pc,
                    )

            # ---------- tail: normalize ----------
            w32 = tailp.tile([B, L], f32)
            nc.sync.dma_start(out=w32, in_=e_dram[:].rearrange("(b l) -> b l", b=B))
            s32 = tailp.tile([B, 1], f32)
            nc.vector.reduce_sum(out=s32, in_=w32, axis=AX.X)
            rs = tailp.tile([B, 1], f32)
            nc.vector.reciprocal(out=rs, in_=s32)
            al = tailp.tile([B, L], f32)
            nc.vector.tensor_scalar_mul(al, w32, rs)
            nc.sync.dma_start(out=alpha_o[:], in_=al)
            nc.vector.tensor_scalar_mul(ctx_sb, ctx_sb, rs)
            nc.sync.dma_start(out=ctx_o[:], in_=ctx_sb)

    nc.finalize()
    return nc


def _get_nc():
    global _NC
    if _NC is None:
        _NC = _build()
    return _NC


def _host_prep(inputs):
    img = np.asarray(inputs["img_features"], dtype=np.float32)
    hid = np.asarray(inputs["hidden_state"], dtype=np.float32)
    Wq = np.asarray(inputs["Wq"], dtype=np.float32)
    bq = np.asarray(inputs["bq"], dtype=np.float32)
    Wk = np.asarray(inputs["Wk"], dtype=np.float32)
    bk = np.asarray(inputs["bk"], dtype=np.float32)
    Wv = np.asarray(inputs["Wv"], dtype=np.float32)

    wqT = np.ascontiguousarray(Wq.T).astype(BF)          # [H, H]
    wkT = np.ascontiguousarray(Wk.T).astype(BF)          # [E, H]
    bqk = (bq + bk).astype(BF)[None, :]                  # [1, H]
    wv_h = np.ascontiguousarray(Wv[0].reshape(HT, 128).T).astype(BF)  # [128, HT]
    ident = np.eye(128, dtype=np.float32).astype(BF)
    rows = np.arange(BL) // L
    ind = (rows[:, None] == np.arange(B)[None, :]).astype(np.float32)
    ind_bf = ind.astype(BF)                              # [BL, B]
    inda = np.concatenate([ind.T, np.ones((1, BL), np.float32)], axis=0).astype(BF)

    in_maps = []
    for i in range(NCORES):
        sl = slice(i * B, (i + 1) * B)
        in_maps.append(
            {
                "x": np.ascontiguousarray(img[sl].reshape(BL, E)),
                "hiddenT": np.ascontiguousarray(hid[sl].T).astype(BF),
                "wqT": wqT,
                "wkT": wkT,
                "bqk": bqk,
                "wv": wv_h,
                "ind": ind_bf,
                "ind_aug": inda,
                "identity": ident,
            }
        )
    return in_maps


def kernel(**inputs):
    global LAST_RESULT
    nc = _get_nc()
    in_maps = _host_prep(inputs)
    res = run_bass_kernel_spmd(nc, in_maps, core_ids=list(range(NCORES)))
    LAST_RESULT = res
    ctx = np.concatenate(
        [np.asarray(res.results[i]["ctx"]) for i in range(NCORES)], axis=0
    ).astype(np.float32)
    alpha = np.concatenate(
        [np.asarray(res.results[i]["alpha"]) for i in range(NCORES)], axis=0
    ).astype(np.float32)
    return ctx, alpha


# revision 11
# speedup vs baseline: 1.2149x; 1.0896x over previous
"""Bahdanau-attention kernel for TRN2, 8 NeuronCores, batch-parallel.

Per core (batch shard b=32):
  x = img_features [6272, 2048] f32 (bl-flattened)
  K_s^T[h, bl]  = sum_e WkT[e,h] * xT[e,bl]          (PE, bf16, xT via PE transpose)
  += (Qh + bq + bk) via indicator matmul              (PE)
  attT = tanh(psum)                                   (ACT, drains psum -> sbuf bf16)
  e[bl] = Wv . attT                                   (PE, M=1)
  w = exp(e)                                          (ACT; softmax shift-invariance
                                                       makes max-subtraction and bv
                                                       unnecessary: |e| <= ~11)
  ctx = sum_bl w[bl] * x[bl, :] (per batch, via indicator-masked lhsT), then / sum(w)
  alpha = w / sum(w)
"""

import sys

for _p in ("/opt/trn_rl_repo",):
    if _p not in sys.path:
        sys.path.insert(0, _p)

import numpy as np
import ml_dtypes

import concourse.bass as bass
import concourse.mybir as mybir
from concourse import bacc
from concourse.tile import TileContext
from concourse.bass_utils import run_bass_kernel_spmd

BF = ml_dtypes.bfloat16
NCORES = 8
BFULL = 256
B, L, E, H = 32, 196, 2048, 512  # per-core batch shard
BL = B * L                        # 6272
NT = BL // 128                    # 49 x row-tiles
CH = 512                          # bl chunk (free dim of K_s matmuls)
NCH = (BL + CH - 1) // CH         # 13 (12 full + 1 tail of 128)
ET = E // 128                     # 16
HT = H // 128                     # 4
EC = E // 512                     # 4 (ctx free chunks)

f32 = mybir.dt.float32
bf16 = mybir.dt.bfloat16
Act = mybir.ActivationFunctionType
AX = mybir.AxisListType

LAST_RESULT = None
_NC = None


def _build():
    nc = bacc.Bacc()
    x = nc.declare_dram_parameter("x", [BL, E], f32, isOutput=False)
    hT = nc.declare_dram_parameter("hiddenT", [H, B], bf16, isOutput=False)
    wqT = nc.declare_dram_parameter("wqT", [H, H], bf16, isOutput=False)
    wkT = nc.declare_dram_parameter("wkT", [E, H], bf16, isOutput=False)
    bqk = nc.declare_dram_parameter("bqk", [1, H], bf16, isOutput=False)
    wv = nc.declare_dram_parameter("wv", [128, HT], bf16, isOutput=False)
    ind = nc.declare_dram_parameter("ind", [BL, B], bf16, isOutput=False)
    inda = nc.declare_dram_parameter("ind_aug", [B + 1, BL], bf16, isOutput=False)
    ident = nc.declare_dram_parameter("identity", [128, 128], bf16, isOutput=False)
    ctx_o = nc.declare_dram_parameter("ctx", [B, E], f32, isOutput=True)
    alpha_o = nc.declare_dram_parameter("alpha", [B, L], f32, isOutput=True)

    with TileContext(nc) as tc:
        with (
            tc.tile_pool(name="singles", bufs=1) as singles,
            tc.tile_pool(name="xb", bufs=12) as xbp,
            tc.tile_pool(name="xT", bufs=2) as xtp,
            tc.tile_pool(name="attT", bufs=2) as atp,
            tc.tile_pool(name="wt", bufs=8) as wtp,
            tc.tile_pool(name="wsl", bufs=2) as wslp,
            tc.tile_pool(name="wcol", bufs=2) as wcolp,
            tc.tile_pool(name="tail", bufs=1) as tailp,
            tc.tile_pool(name="ppt", bufs=2, space="PSUM") as ppt,
            tc.tile_pool(name="ppk", bufs=2, space="PSUM") as ppk,
            tc.tile_pool(name="ppe", bufs=2, space="PSUM") as ppe,
            tc.tile_pool(name="ppc", bufs=2, space="PSUM") as ppc,
            tc.tile_pool(name="dram", bufs=1, space="DRAM") as dramp,
        ):
            # ---------- constants (small / critical-path first: the sync queue
            # drains slowly against the 1MB x-load packets, so queue order
            # decides when PE can start) ----------
            id_sb = singles.tile([128, 128], bf16)
            nc.sync.dma_start(out=id_sb, in_=ident[:])
            wv_sb = singles.tile([128, HT], bf16)
            nc.sync.dma_start(out=wv_sb, in_=wv[:])
            hT_sb = singles.tile([128, HT, B], bf16)
            nc.sync.dma_start(out=hT_sb, in_=hT[:].rearrange("(t p) b -> p t b", p=128))
            wqT_sb = singles.tile([128, HT, H], bf16)
            nc.sync.dma_start(out=wqT_sb, in_=wqT[:].rearrange("(t p) h -> p t h", p=128))
            ind_sb = singles.tile([128, NT, B], bf16)
            nc.sync.dma_start(out=ind_sb, in_=ind[:].rearrange("(k p) b -> p k b", p=128))
            inda_sb = singles.tile([B + 1, BL], bf16)
            nc.sync.dma_start(out=inda_sb, in_=inda[:])
            wkT_sb = singles.tile([128, ET, H], bf16)
            nc.sync.dma_start(out=wkT_sb, in_=wkT[:].rearrange("(k p) h -> p k h", p=128))

            ctx_sb = singles.tile([B, E], f32)
            nc.vector.memset(ctx_sb, 0.0)
            e_dram = dramp.tile([BL], f32)

            # ---------- Q_h = hidden @ Wq.T (+ bq + bk), layout [b, h] ----------
            psq = ppc.tile([B, 512], f32, tag="pc")
            for t in range(HT):
                nc.tensor.matmul(
                    psq,
                    lhsT=hT_sb[:, t, :],
                    rhs=wqT_sb[:, t, :],
                    start=(t == 0),
                    stop=(t == HT - 1),
                )
            qb_sb = singles.tile([B + 1, H], bf16)
            nc.scalar.copy(out=qb_sb[0:B, :], in_=psq[:, :])
            nc.sync.dma_start(out=qb_sb[B : B + 1, :], in_=bqk[:])

            # ---------- main streaming loop over bl chunks ----------
            for c in range(NCH):
                n0 = c * CH
                n1 = min(BL, n0 + CH)
                n = n1 - n0
                kt = n // 128  # x-tiles in this chunk (4, tail: 1)

                xb = []
                for i in range(kt):
                    t_ = xbp.tile([128, E], bf16, tag="xb")
                    nc.gpsimd.dma_start(
                        out=t_, in_=x[n0 + 128 * i : n0 + 128 * (i + 1), :]
                    )  # f32 -> bf16 cast in SWDGE
                    xb.append(t_)

                # transpose x chunk: xT[e_part, k, bl]. Regular matmul
                # (out = x_tile.T @ I) instead of transpose-mode: pipelines
                # with LDWEIGHTS and keeps the PE clock warm; products are
                # exact (x1.0, f32 psum).
                xT_c = xtp.tile([128, ET, CH], bf16, tag="xT")
                for k in range(ET):
                    pt = ppt.tile([128, CH], f32, tag="pt")
                    for i in range(kt):
                        nc.tensor.matmul(
                            pt[:, 128 * i : 128 * (i + 1)],
                            lhsT=xb[i][:, 128 * k : 128 * (k + 1)],
                            rhs=id_sb,
                        )
                    # drain psum -> sbuf; split between DVE and ACT
                    if k % 3 == 0:
                        nc.scalar.copy(out=xT_c[:, k, :n], in_=pt[:, :n])
                    else:
                        nc.vector.tensor_copy(out=xT_c[:, k, :n], in_=pt[:, :n])

                # K_s^T chunk + QB + tanh
                attT_c = atp.tile([128, HT, CH], bf16, tag="attT")
                for j in range(HT):
                    pk = ppk.tile([128, CH], f32, tag="pk")
                    for k in range(ET):
                        nc.tensor.matmul(
                            pk[:, :n],
                            lhsT=wkT_sb[:, k, 128 * j : 128 * (j + 1)],
                            rhs=xT_c[:, k, :n],
                            start=(k == 0),
                            stop=False,
                        )
                    nc.tensor.matmul(
                        pk[:, :n],
                        lhsT=qb_sb[:, 128 * j : 128 * (j + 1)],
                        rhs=inda_sb[:, n0:n1],
                        start=False,
                        stop=True,
                    )
                    nc.scalar.activation(
                        out=attT_c[:, j, :n], in_=pk[:, :n], func=Act.Tanh
                    )

                # e = Wv . attT ; w = exp(e)
                pe_ = ppe.tile([1, CH], f32, tag="pe")
                for j in range(HT):
                    nc.tensor.matmul(
                        pe_[:, :n],
                        lhsT=wv_sb[:, j : j + 1],
                        rhs=attT_c[:, j, :n],
                        start=(j == 0),
                        stop=(j == HT - 1),
                    )
                wsl = wslp.tile([1, CH], f32, tag="wsl")
                nc.scalar.activation(out=wsl[:, :n], in_=pe_[:, :n], func=Act.Exp)
                nc.sync.dma_start(out=e_dram[n0:n1], in_=wsl[:, :n])

                # w back as per-tile partition columns
                wcol_c = wcolp.tile([128, 4], f32, tag="wcol")
                nc.sync.dma_start(
                    out=wcol_c[:, :kt],
                    in_=e_dram[n0:n1].rearrange("(k p) -> p k", p=128),
                )

                # ctx += sum_bl w * x   (indicator-masked lhsT, M=32)
                wts = []
                for i in range(kt):
                    kg = n0 // 128 + i
                    wt_i = wtp.tile([128, B], bf16, tag="wt")
                    nc.vector.tensor_scalar_mul(
                        wt_i, ind_sb[:, kg, :], wcol_c[:, i : i + 1]
                    )
                    wts.append(wt_i)
                for ec in range(EC):
                    pc = ppc.tile([B, 512], f32, tag="pc")
                    for i in range(kt):
                        nc.tensor.matmul(
                            pc,
                            lhsT=wts[i],
                            rhs=xb[i][:, 512 * ec : 512 * (ec + 1)],
                            start=(i == 0),
                            stop=(i == kt - 1),
                        )
                    nc.vector.tensor_add(
                        ctx_sb[:, 512 * ec : 512 * (ec + 1)],
                        ctx_sb[:, 512 * ec : 512 * (ec + 1)],
                        pc,
                    )

            # ---------- tail: normalize ----------
            w32 = tailp.tile([B, L], f32)
            nc.sync.dma_start(out=w32, in_=e_dram[:].rearrange("(b l) -> b l", b=B))
            s32 = tailp.tile([B, 1], f32)
            nc.vector.reduce_sum(out=s32, in_=w32, axis=AX.X)
            rs = tailp.tile([B, 1], f32)
            nc.vector.reciprocal(out=rs, in_=s32)
            al = tailp.tile([B, L], f32)
            nc.vector.tensor_scalar_mul(al, w32, rs)
            nc.sync.dma_start(out=alpha_o[:], in_=al)
            nc.vector.tensor_scalar_mul(ctx_sb, ctx_sb, rs)
            nc.sync.dma_start(out=ctx_o[:], in_=ctx_sb)

    nc.finalize()
    return nc


def _get_nc():
    global _NC
    if _NC is None:
        _NC = _build()
    return _NC


def _host_prep(inputs):
    img = np.asarray(inputs["img_features"], dtype=np.float32)
    hid = np.asarray(inputs["hidden_state"], dtype=np.float32)
    Wq = np.asarray(inputs["Wq"], dtype=np.float32)
    bq = np.asarray(inputs["bq"], dtype=np.float32)
    Wk = np.asarray(inputs["Wk"], dtype=np.float32)
    bk = np.asarray(inputs["bk"], dtype=np.float32)
    Wv = np.asarray(inputs["Wv"], dtype=np.float32)

    wqT = np.ascontiguousarray(Wq.T).astype(BF)          # [H, H]
    wkT = np.ascontiguousarray(Wk.T).astype(BF)          # [E, H]
    bqk = (bq + bk).astype(BF)[None, :]                  # [1, H]
    wv_h = np.ascontiguousarray(Wv[0].reshape(HT, 128).T).astype(BF)  # [128, HT]
    ident = np.eye(128, dtype=np.float32).astype(BF)
    rows = np.arange(BL) // L
    ind = (rows[:, None] == np.arange(B)[None, :]).astype(np.float32)
    ind_bf = ind.astype(BF)                              # [BL, B]
    inda = np.concatenate([ind.T, np.ones((1, BL), np.float32)], axis=0).astype(BF)

    in_maps = []
    for i in range(NCORES):
        sl = slice(i * B, (i + 1) * B)
        in_maps.append(
            {
                "x": np.ascontiguousarray(img[sl].reshape(BL, E)),
                "hiddenT": np.ascontiguousarray(hid[sl].T).astype(BF),
                "wqT": wqT,
                "wkT": wkT,
                "bqk": bqk,
                "wv": wv_h,
                "ind": ind_bf,
                "ind_aug": inda,
                "identity": ident,
            }
        )
    return in_maps


def kernel(**inputs):
    global LAST_RESULT
    nc = _get_nc()
    in_maps = _host_prep(inputs)
    res = run_bass_kernel_spmd(nc, in_maps, core_ids=list(range(NCORES)))
    LAST_RESULT = res
    ctx = np.concatenate(
        [np.asarray(res.results[i]["ctx"]) for i in range(NCORES)], axis=0
    ).astype(np.float32)
    alpha = np.concatenate(
        [np.asarray(res.results[i]["alpha"]) for i in range(NCORES)], axis=0
    ).astype(np.float32)
    return ctx, alpha
